# revision 25
# baseline (speedup 1.0000x reference)
"""Trainium2 Bass kernel for nn_MATAPCell (GRU + single-query MHA over per-row
memory + gated blend + memory shift-write).

Contract: kernel(**inputs) takes FULL unsharded fp32 inputs (see shapes below),
shards batch across 8 NeuronCores (pure data parallel, weights replicated),
runs a Bass/Tile kernel per core, and gathers the full outputs.

Returns (h_corr [B,256] f32, new_memory_flat [B,2560] f32) matching reference.

Dataflow per core (BS=1024 rows, two column-groups of 512):
  - activations live TRANSPOSED in SBUF: [feature(partitions), row(free)]
  - natural-layout rows are DMA'd in and PE-transposed (f32) into bf16 tiles
  - all matmuls bf16 (weights host-cast), fp32 PSUM accumulate
  - attention: k = mem@Wk on PE; scores q.k via DVE mult + PE mask-matmul
    partition reduction; softmax (no max-subtract: |scores| < 0.3);
    apply: v on PE, attn replicated across dk-partitions via PE mask-matmul,
    DVE mult, wide-AP tree accumulation
  - h-path carrier (h_prev -> gru_out -> blend -> LN2 -> h_corr) kept f32
    in elementwise ops for accuracy
  - memory shift is a pure SBUF->DRAM f32 copy of the loaded mem tile
  - layernorms computed in T-layout via ones-matmul partition sums + rank-1
    replication matmuls
"""

import sys
import os
import numpy as np

for _p in ("/opt/trn_rl_repo",):
    if os.path.isdir(_p) and _p not in sys.path:
        sys.path.insert(0, _p)

import ml_dtypes
from contextlib import ExitStack

import concourse.bass as bass
import concourse.tile as tile
from concourse import bacc, mybir
from concourse.bass_utils import run_bass_kernel_spmd

BF16 = ml_dtypes.bfloat16
FP32 = mybir.dt.float32
BF = mybir.dt.bfloat16

B, D, M, H, DK = 8192, 256, 10, 4, 64
N_CORES = 8
BS = B // N_CORES            # 1024 rows per core
NG = BS // 512               # column groups per core (512 cols each)
GC = 512                     # cols per group
NR4 = 4                      # row-tiles (128 rows) per group
HKD = H * DK                 # 256
D3 = 3 * D                   # 768
LN_EPS = 1e-3
ALU = mybir.AluOpType
ACTF = mybir.ActivationFunctionType

_BUILD_CACHE = {}


def _blob_layout():
    """(name, rows, cols) entries for the bf16 and f32 constant blobs."""
    b = []
    for nm, cols in [("w_zr_x", 512), ("w_zr_h", 512), ("w_h_x", 256),
                     ("w_hh", 256), ("w_ne", 256), ("w_q", 256), ("w_k", 256),
                     ("w_v", 256), ("w_o", 256), ("w_ctx", 256)]:
        b.append((nm + "_0", 128, cols))
        b.append((nm + "_1", 128, cols))
    for k in range(4):
        b.append((f"w_gate_{k}", 128, 256))
    b += [("neg_gw", 1, 256), ("g_out_row", 1, 256),
          ("neg_beta_out_row", 1, 256), ("ones_row", 1, 512),
          ("ones_1_128", 1, 128), ("ones_128_1", 128, 1),
          ("summask", 40, 4), ("replmask", 4, 40)]
    for m in range(M):
        for t in range(2):
            b.append((f"smask_{m}_{t}", 128, 40))
            b.append((f"amask_{m}_{t}", 40, 128))
    f = [("vecs_0", 128, NVEC), ("vecs_1", 128, NVEC)]
    for k in range(4):
        f.append((f"bzr_{k}", 128, 1))
    f.append(("ident", 128, 128))
    return b, f


def _blob_offsets():
    b, f = _blob_layout()
    bo, off = {}, 0
    for nm, rows, cols in b:
        bo[nm] = (off, rows, cols)
        off += cols
    bcols = off
    fo, off = {}, 0
    for nm, rows, cols in f:
        fo[nm] = (off, rows, cols)
        off += cols
    return bo, bcols, fo, off


VEC = {"b_xh2": 0, "b_hh": 1, "bq_s": 2, "bo_p": 3, "bw_ctx2": 4,
       "b_gate": 5, "b_ne": 6, "eps_dd": 7}
NVEC = 8
BOFF, BCOLS, FOFF, FCOLS = _blob_offsets()


def _prep_weights(inp):
    """Host-side weight fusion + bf16 casts + blob packing. All small."""
    f = lambda x: np.asarray(x, np.float32)
    W_in = f(inp["W_in"]); b_in = f(inp["b_in"])
    gru_k = f(inp["gru_k"]); gru_rk = f(inp["gru_rk"]); gru_b = f(inp["gru_b"])
    Wq = f(inp["Wq"]).reshape(D, HKD); bq = f(inp["bq"]).reshape(HKD)
    Wk = f(inp["Wk"]).reshape(D, HKD)
    Wv = f(inp["Wv"]).reshape(D, HKD); bv = f(inp["bv"]).reshape(HKD)
    Wo = f(inp["Wo"]).reshape(HKD, D); bo = f(inp["bo"])
    g_attn = f(inp["g_attn"]); beta_attn = f(inp["beta_attn"])
    g_out = f(inp["g_out"]); beta_out = f(inp["beta_out"])
    W_ctx = f(inp["W_ctx"]); b_ctx = f(inp["b_ctx"])
    W_gate = f(inp["W_gate"]); b_gate = f(inp["b_gate"])
    W_mem = f(inp["W_mem"]); b_mem = f(inp["b_mem"])

    t = {}
    for nm, arr in [("w_zr_x", W_in @ gru_k[:, :2 * D]),
                    ("w_zr_h", gru_rk[:, :2 * D]),
                    ("w_h_x", W_in @ gru_k[:, 2 * D:]),
                    ("w_hh", gru_rk[:, 2 * D:]),
                    ("w_ne", W_in @ W_mem), ("w_q", Wq), ("w_k", Wk),
                    ("w_v", Wv), ("w_o", Wo),
                    ("w_ctx", g_attn[:, None] * W_ctx)]:
        t[nm + "_0"] = arr[0:128]
        t[nm + "_1"] = arr[128:256]
    for k in range(4):
        t[f"w_gate_{k}"] = W_gate[k * 128:(k + 1) * 128]
    gw = g_attn @ W_ctx
    t["neg_gw"] = -(gw / np.float32(D))[None, :]
    t["g_out_row"] = g_out[None, :]
    t["neg_beta_out_row"] = -beta_out[None, :]
    t["ones_row"] = np.ones((1, 512), np.float32)
    t["ones_1_128"] = np.ones((1, 128), np.float32)
    t["ones_128_1"] = np.ones((128, 1), np.float32)
    summask = np.zeros((H * M, H), np.float32)
    for h in range(H):
        summask[h * M:(h + 1) * M, h] = 1.0
    t["summask"] = summask
    repl = np.zeros((H, H * M), np.float32)
    for h in range(H):
        repl[h, h * M:(h + 1) * M] = 1.0
    t["replmask"] = repl
    for m in range(M):
        for tt in range(2):
            sm = np.zeros((128, H * M), np.float32)
            am = np.zeros((H * M, 128), np.float32)
            for p in range(128):
                h = 2 * tt + p // DK
                sm[p, h * M + m] = 1.0
                am[h * M + m, p] = 1.0
            t[f"smask_{m}_{tt}"] = sm
            t[f"amask_{m}_{tt}"] = am

    kb = np.zeros((128, BCOLS), np.float32)
    for nm, (off, rows, cols) in BOFF.items():
        kb[:rows, off:off + cols] = t[nm]
    kb = kb.astype(BF16)

    b_zr = gru_b[0, :2 * D] + gru_b[1, :2 * D] + b_in @ gru_k[:, :2 * D]
    b_xh = gru_b[0, 2 * D:] + b_in @ gru_k[:, 2 * D:]
    b_hh = gru_b[1, 2 * D:]
    bq_s = bq / np.float32(np.sqrt(DK))
    bo_p = bv @ Wo + bo
    bw_ctx = beta_attn @ W_ctx + b_ctx
    b_ne = b_in @ W_mem + b_mem
    eps_col = np.full((D,), np.float32(D) * np.float32(D) * np.float32(LN_EPS),
                      np.float32)
    vecs = np.stack([2.0 * b_xh, b_hh, bq_s, bo_p, 2.0 * bw_ctx, b_gate,
                     b_ne, eps_col], axis=1)
    tf = {"vecs_0": vecs[0:128], "vecs_1": vecs[128:256],
          "ident": np.eye(128, dtype=np.float32)}
    for k in range(4):
        tf[f"bzr_{k}"] = b_zr[k * 128:(k + 1) * 128, None]
    kf = np.zeros((128, FCOLS), np.float32)
    for nm, (off, rows, cols) in FOFF.items():
        kf[:rows, off:off + cols] = tf[nm]
    return {"kb": np.ascontiguousarray(kb), "kf": np.ascontiguousarray(kf)}


def build_kernel():
    nc = bacc.Bacc("TRN2", target_bir_lowering=False, debug=False,
                   num_devices=N_CORES)

    def din(name, shape, dt=FP32):
        return nc.dram_tensor(name, shape, dt, kind="ExternalInput").ap()

    def dout(name, shape, dt=FP32):
        return nc.dram_tensor(name, shape, dt, kind="ExternalOutput").ap()

    x_d = din("x", (BS, D))
    h_d = din("h", (BS, D))
    mem_d = din("mem", (BS, M * D))
    kb_d = din("kb", (128, BCOLS), BF)
    kf_d = din("kf", (128, FCOLS))
    hc_d = dout("hcorr", (BS, D))
    nm_d = dout("newmem", (BS, M * D))

    with tile.TileContext(nc) as tc, ExitStack() as ctx:
        konst = ctx.enter_context(tc.tile_pool(name="konst", bufs=1))
        nat = ctx.enter_context(tc.tile_pool(name="nat", bufs=1))
        big = ctx.enter_context(tc.tile_pool(name="big", bufs=1))
        act = ctx.enter_context(tc.tile_pool(name="act", bufs=1))
        sml = ctx.enter_context(tc.tile_pool(name="sml", bufs=1))
        pst = ctx.enter_context(tc.tile_pool(name="pst", bufs=1, space="PSUM"))

        def ps_tile(name, shape=None, tag="ps", bufs=3):
            return pst.tile(shape or [128, GC], FP32, tag=tag, bufs=bufs,
                            name=name)

        def pml_tile(name):
            return ps_tile(name, tag="pml", bufs=4)

        # ---- constant blobs: 2 DMAs total (kf first: holds the transpose
        # identity; weights (kb) aren't needed until GRU) ----
        kf_sb = konst.tile([128, FCOLS], FP32, tag="kf", name="kf_sb")
        nc.sync.dma_start(kf_sb[:, :], kf_d[:, :])
        kb_sb = konst.tile([128, BCOLS], BF, tag="kb", name="kb_sb")

        def KB(nm):
            off, rows, cols = BOFF[nm]
            return kb_sb[0:rows, off:off + cols]

        def KF(nm):
            off, rows, cols = FOFF[nm]
            return kf_sb[0:rows, off:off + cols]

        W = {nm: [KB(nm + "_0"), KB(nm + "_1")]
             for nm in ("w_zr_x", "w_zr_h", "w_h_x", "w_hh", "w_ne", "w_q",
                        "w_k", "w_v", "w_o", "w_ctx")}
        W["w_gate"] = [KB(f"w_gate_{k}") for k in range(4)]
        smask_sb = [[KB(f"smask_{m}_{t}") for t in range(2)] for m in range(M)]
        amask_sb = [[KB(f"amask_{m}_{t}") for t in range(2)] for m in range(M)]
        summask_sb = KB("summask")
        replmask_sb = KB("replmask")
        ones_128_1 = KB("ones_128_1")
        ones_1_128 = KB("ones_1_128")
        ones_row = KB("ones_row")
        vecs_sb = [KF("vecs_0"), KF("vecs_1")]
        bzr_sb = [KF(f"bzr_{k}") for k in range(4)]
        ident = KF("ident")
        W["neg_gw"] = KB("neg_gw")
        W["g_out_row"] = KB("g_out_row")
        W["neg_beta_out_row"] = KB("neg_beta_out_row")

        def vbias(col, kt):
            return vecs_sb[kt][:, VEC[col]:VEC[col] + 1]

        # ---------------- per-group pipeline, phase-interleaved ----------------
        ST = [dict() for _ in range(NG)]

        def preload(g):
            s = ST[g]
            rows = slice(g * GC, (g + 1) * GC)
            mn = nat.tile([128, NR4 * M * D], FP32, tag="memnat", name="mn")
            for r4 in range(NR4):
                r = NR4 * g + r4
                nc.sync.dma_start(mn[:, r4 * M * D:(r4 + 1) * M * D],
                                  mem_d[r * 128:(r + 1) * 128, :])
            xn = nat.tile([128, NR4 * D], FP32, tag="xnat", bufs=2, name="xn")
            nc.sync.dma_start(
                xn[:, :], x_d[rows, :].rearrange("(a p) c -> p a c", p=128))
            hn = nat.tile([128, NR4 * D], FP32, tag="hnat", bufs=2, name="hn")
            nc.sync.dma_start(
                hn[:, :], h_d[rows, :].rearrange("(a p) c -> p a c", p=128))
            # memory shift store (scalar HWDGE queue; won't stall loads)
            nc.scalar.dma_start(
                nm_d[rows, 0:(M - 1) * D].rearrange("(a p) c -> p a c", p=128),
                mn.rearrange("p (a c) -> p a c", a=NR4)[:, :, D:M * D])
            s.update(xn=xn, hn=hn, mn=mn)

        def phase_T(g):
            s = ST[g]
            xn, hn, mn = s["xn"], s["hn"], s["mn"]

            def transpose_in(src_tile, stride, col_off, dst, evac):
                ps = ps_tile("ps_tr")
                for r4 in range(NR4):
                    o = r4 * stride + col_off
                    nc.tensor.transpose(ps[:, r4 * 128:(r4 + 1) * 128],
                                        src_tile[:, o:o + 128], ident)
                if evac == "scalar":
                    nc.scalar.copy(dst[:, :], ps[:, :])
                else:
                    nc.vector.tensor_copy(dst[:, :], ps[:, :])

            memT = []
            for m in range(M):
                row = []
                for t in range(2):
                    mt = big.tile([128, GC], BF, tag=f"memT_{m}_{t}",
                                  name=f"memT_{m}_{t}")
                    transpose_in(mn, M * D, m * D + t * 128, mt, "scalar")
                    row.append(mt)
                memT.append(row)
            xT = []
            for t in range(2):
                xt = act.tile([128, GC], BF, tag=f"xT{t}", bufs=2,
                              name=f"xT{t}")
                transpose_in(xn, D, t * 128, xt, "scalar")
                xT.append(xt)
            hT, hTf = [], []
            for t in range(2):
                ps = ps_tile("ps_trh")
                for r4 in range(NR4):
                    o = r4 * D + t * 128
                    nc.tensor.transpose(ps[:, r4 * 128:(r4 + 1) * 128],
                                        hn[:, o:o + 128], ident)
                ht = act.tile([128, GC], BF, tag=f"hT{t}", bufs=2,
                              name=f"hT{t}")
                nc.scalar.copy(ht[:, :], ps[:, :])
                htf = act.tile([128, GC], FP32, tag=f"hTf{t}", name=f"hTf{t}")
                nc.vector.tensor_copy(htf[:, :], ps[:, :])
                hT.append(ht)
                hTf.append(htf)
            s.update(memT=memT, xT=xT, hT=hT, hTf=hTf)

        def mm_pair(lhs_tiles, out_slice, rhs_tiles, psum_tile,
                    start=True, stop=True):
            nkt = len(lhs_tiles)
            for kt in range(nkt):
                nc.tensor.matmul(
                    psum_tile[:, :], lhs_tiles[kt][:, out_slice],
                    rhs_tiles[kt][:, :],
                    start=(start and kt == 0), stop=(stop and kt == nkt - 1))

        def phase_GRU(g):
            s = ST[g]
            xT, hT, hTf = s["xT"], s["hT"], s["hTf"]
            zr = [None] * 4
            for mt in (2, 3, 0, 1):   # r gates first: the hc chain needs r
                ps = ps_tile(f"ps_zr{mt}")
                sl = slice(mt * 128, (mt + 1) * 128)
                mm_pair(W["w_zr_x"], sl, xT, ps, start=True, stop=False)
                mm_pair(W["w_zr_h"], sl, hT, ps, start=False, stop=True)
                zr_t = act.tile([128, GC], FP32, tag=f"zr{mt}", name=f"zr{mt}")
                nc.scalar.activation(zr_t[:, :], ps[:, :], ACTF.Sigmoid,
                                     bias=bzr_sb[mt][:, :], scale=1.0)
                zr[mt] = zr_t
            z_f, r_f = zr[0:2], zr[2:4]

            hc_f, gru_f, gru_b16 = [], [], []
            for mt in range(2):
                sl = slice(mt * 128, (mt + 1) * 128)
                ps_xh = ps_tile(f"ps_xh{mt}")
                mm_pair(W["w_h_x"], sl, xT, ps_xh)
                ps_hh = ps_tile(f"ps_hh{mt}")
                mm_pair(W["w_hh"], sl, hT, ps_hh)
                t1 = act.tile([128, GC], FP32, tag="fscr", bufs=3,
                              name=f"t1_{mt}")
                nc.vector.scalar_tensor_tensor(t1[:, :], ps_hh[:, :],
                                               vbias("b_hh", mt),
                                               r_f[mt][:, :],
                                               op0=ALU.add, op1=ALU.mult)
                t2 = act.tile([128, GC], FP32, tag="fscr", bufs=3,
                              name=f"t2_{mt}")
                nc.vector.tensor_tensor(t2[:, :], t1[:, :], ps_xh[:, :],
                                        ALU.add)
                sgm = act.tile([128, GC], FP32, tag="fscr", bufs=3,
                               name=f"sgm{mt}")
                nc.scalar.activation(sgm[:, :], t2[:, :], ACTF.Sigmoid,
                                     bias=vbias("b_xh2", mt), scale=2.0)
                hc = act.tile([128, GC], FP32, tag=f"hc{mt}", name=f"hc{mt}")
                nc.vector.tensor_scalar(hc[:, :], sgm[:, :], 2.0, -1.0,
                                        op0=ALU.mult, op1=ALU.add)
                hc_f.append(hc)
                d1 = act.tile([128, GC], FP32, tag="fscr", bufs=3,
                              name=f"d1_{mt}")
                nc.vector.tensor_tensor(d1[:, :], hTf[mt][:, :], hc[:, :],
                                        ALU.subtract)
                e1 = act.tile([128, GC], FP32, tag="fscr", bufs=3,
                              name=f"e1_{mt}")
                nc.vector.tensor_tensor(e1[:, :], z_f[mt][:, :], d1[:, :],
                                        ALU.mult)
                gr = act.tile([128, GC], FP32, tag=f"gru{mt}", name=f"gru{mt}")
                nc.vector.tensor_tensor(gr[:, :], hc[:, :], e1[:, :], ALU.add)
                gru_f.append(gr)
                grb = act.tile([128, GC], BF, tag=f"grub{mt}", name=f"grub{mt}")
                nc.vector.tensor_copy(grb[:, :], gr[:, :])
                gru_b16.append(grb)

            q_sb = []
            for mt in range(2):
                sl = slice(mt * 128, (mt + 1) * 128)
                ps = ps_tile(f"ps_q{mt}")
                mm_pair(W["w_q"], sl, gru_b16, ps)
                qs = act.tile([128, GC], BF, tag=f"q{mt}", name=f"q{mt}")
                nc.scalar.activation(qs[:, :], ps[:, :], ACTF.Identity,
                                     bias=vbias("bq_s", mt),
                                     scale=float(1.0 / np.sqrt(DK)))
                q_sb.append(qs)
            s.update(gru_f=gru_f, gru_b16=gru_b16, q_sb=q_sb)

        def phase_L1(g):
            s = ST[g]
            memT, q_sb = s["memT"], s["q_sb"]
            ps_sc = pst.tile([H * M, GC], FP32, tag="psc", bufs=1,
                             name="ps_scores")
            # software-pipelined emission: k-matmuls for m+1 are emitted
            # before the DVE mult / mask-matmul of m, so the PE stream never
            # head-of-line blocks on the DVE product.
            ps_k = {}
            for t in range(2):
                sl = slice(t * 128, (t + 1) * 128)
                ps_k[(0, t)] = pml_tile(f"ps_k0{t}")
                mm_pair(W["w_k"], sl, memT[0], ps_k[(0, t)])
            for m in range(M):
                if m + 1 < M:
                    for t in range(2):
                        sl = slice(t * 128, (t + 1) * 128)
                        ps_k[(m + 1, t)] = pml_tile(f"ps_k{m + 1}{t}")
                        mm_pair(W["w_k"], sl, memT[m + 1], ps_k[(m + 1, t)])
                for t in range(2):
                    prod = act.tile([128, GC], BF, tag=f"sprod{t}",
                                    name=f"sprod{m}{t}")
                    nc.vector.tensor_tensor(prod[:, :], q_sb[t][:, :],
                                            ps_k[(m, t)][:, :], ALU.mult)
                    nc.tensor.matmul(ps_sc[:, :], smask_sb[m][t],
                                     prod[:, :],
                                     start=(m == 0 and t == 0),
                                     stop=(m == M - 1 and t == 1))
            s["ps_sc"] = ps_sc

        def phase_SM(g):
            s = ST[g]
            ps_sc, xT = s["ps_sc"], s["xT"]
            e_sb = act.tile([H * M, GC], BF, tag="e_sb", name="e_sb")
            nc.scalar.activation(e_sb[:, :], ps_sc[:, :], ACTF.Exp)
            ps_sum = ps_tile("ps_sum", [H, GC])
            nc.tensor.matmul(ps_sum[:, :], summask_sb[:, :], e_sb[:, :],
                             start=True, stop=True)
            rec_f = sml.tile([H, GC], FP32, tag="lnscr", bufs=2, name="rec_f")
            nc.vector.reciprocal_approx_fast(rec_f[:, :], ps_sum[:, :])
            rec_sb = sml.tile([H, GC], BF, tag="rec", name="rec_sb")
            nc.vector.tensor_copy(rec_sb[:, :], rec_f[:, :])
            ps_rr = ps_tile("ps_rr", [H * M, GC])
            nc.tensor.matmul(ps_rr[:, :], replmask_sb[:, :], rec_sb[:, :],
                             start=True, stop=True)
            attn_sb = act.tile([H * M, GC], BF, tag="attn", name="attn_sb")
            nc.vector.tensor_tensor(attn_sb[:, :], e_sb[:, :], ps_rr[:, :],
                                    ALU.mult)
            # new_entry here: PE filler during softmax
            neT = []
            for mt in range(2):
                sl = slice(mt * 128, (mt + 1) * 128)
                ps = ps_tile(f"ps_ne{mt}")
                mm_pair(W["w_ne"], sl, xT, ps)
                ne = act.tile([128, GC], FP32, tag=f"neT{mt}", name=f"neT{mt}")
                nc.scalar.activation(ne[:, :], ps[:, :], ACTF.Identity,
                                     bias=vbias("b_ne", mt), scale=1.0)
                neT.append(ne)
            s.update(attn_sb=attn_sb, neT=neT)

        def phase_L2(g):
            s = ST[g]
            memT, attn_sb = s["memT"], s["attn_sb"]
            MC = 2 * GC
            slab = big.tile([128, 5 * MC], BF, tag="pslab", name="pslab")
            ps_v = {}
            for t in range(2):
                sl = slice(t * 128, (t + 1) * 128)
                ps_v[(0, t)] = pml_tile(f"ps_v0{t}")
                mm_pair(W["w_v"], sl, memT[0], ps_v[(0, t)])
            tmps = {}
            for m in range(M):
                if m >= 5:
                    tmps[m] = big.tile([128, MC], BF, tag="ptmp", bufs=2,
                                       name=f"ptmp{m}")
                if m + 1 < M:
                    for t in range(2):
                        sl = slice(t * 128, (t + 1) * 128)
                        ps_v[(m + 1, t)] = pml_tile(f"ps_v{m + 1}{t}")
                        mm_pair(W["w_v"], sl, memT[m + 1], ps_v[(m + 1, t)])
                for t in range(2):
                    ps_er = ps_tile(f"ps_er{m}{t}")
                    nc.tensor.matmul(ps_er[:, :], amask_sb[m][t],
                                     attn_sb[:, :], start=True, stop=True)
                    er_sb = sml.tile([128, GC], BF, tag="er", bufs=1,
                                     name=f"er{m}{t}")
                    nc.scalar.copy(er_sb[:, :], ps_er[:, :])
                    dst = (slab[:, m * MC + t * GC:m * MC + (t + 1) * GC]
                           if m < 5 else tmps[m][:, t * GC:(t + 1) * GC])
                    nc.vector.tensor_tensor(dst, er_sb[:, :],
                                            ps_v[(m, t)][:, :], ALU.mult)
                if m >= 5:
                    c = (m - 5) * MC
                    nc.vector.tensor_tensor(slab[:, c:c + MC],
                                            slab[:, c:c + MC],
                                            tmps[m][:, :], ALU.add)
            nc.vector.tensor_tensor(slab[:, 0:2 * MC], slab[:, 0:2 * MC],
                                    slab[:, 2 * MC:4 * MC], ALU.add)
            nc.vector.tensor_tensor(slab[:, 0:MC], slab[:, 0:MC],
                                    slab[:, MC:2 * MC], ALU.add)
            nc.vector.tensor_tensor(slab[:, 0:MC], slab[:, 0:MC],
                                    slab[:, 4 * MC:5 * MC], ALU.add)
            U_sb = []
            for t in range(2):
                u_t = act.tile([128, GC], BF, tag=f"U{t}", name=f"U{t}")
                nc.vector.tensor_copy(u_t[:, :], slab[:, t * GC:(t + 1) * GC])
                U_sb.append(u_t)
            s["U_sb"] = U_sb

        def ln_stats(x_b16_tiles, sq_tag):
            ps_s1 = ps_tile("ps_s1_" + sq_tag, [1, GC])
            s1b = sml.tile([1, GC], BF, tag="s1b", bufs=2,
                           name="s1b_" + sq_tag)
            for kt in range(2):
                nc.tensor.matmul(ps_s1[:, :], ones_128_1[:, :],
                                 x_b16_tiles[kt][:, :],
                                 start=(kt == 0), stop=(kt == 1))
            sq = [act.tile([128, GC], BF, tag=f"sq{kt}",
                           name=f"{sq_tag}{kt}") for kt in range(2)]
            for kt in range(2):
                nc.vector.tensor_tensor(sq[kt][:, :], x_b16_tiles[kt][:, :],
                                        x_b16_tiles[kt][:, :], ALU.mult)
            ps_s2 = ps_tile("ps_s2_" + sq_tag, [1, GC])
            for kt in range(2):
                nc.tensor.matmul(ps_s2[:, :], ones_128_1[:, :],
                                 sq[kt][:, :], start=(kt == 0), stop=(kt == 1))
            nc.vector.tensor_copy(s1b[:, :], ps_s1[:, :])
            s1sq = sml.tile([1, GC], FP32, tag="lnscr", bufs=2,
                            name="s1sq_" + sq_tag)
            nc.vector.tensor_tensor(s1sq[:, :], s1b[:, :], s1b[:, :],
                                    ALU.mult)
            var_t = sml.tile([1, GC], FP32, tag="lnscr", bufs=2,
                             name="var_" + sq_tag)
            nc.vector.scalar_tensor_tensor(var_t[:, :], ps_s2[:, :],
                                           float(D), s1sq[:, :],
                                           op0=ALU.mult, op1=ALU.subtract)
            sd = sml.tile([1, GC], FP32, tag="lnscr", bufs=2,
                          name="sd_" + sq_tag)
            nc.scalar.activation(sd[:, :], var_t[:, :], ACTF.Sqrt,
                                 bias=vecs_sb[0][0:1, VEC["eps_dd"]:
                                                 VEC["eps_dd"] + 1],
                                 scale=1.0)
            rc = sml.tile([1, GC], FP32, tag="lnscr", bufs=2,
                          name="rc_" + sq_tag)
            nc.vector.reciprocal_approx_fast(rc[:, :], sd[:, :])
            A_b = sml.tile([1, GC], BF, tag="A_b", name="A_" + sq_tag)
            with nc.allow_low_precision("LN scale bf16"):
                nc.vector.tensor_scalar(A_b[:, :], rc[:, :], float(D), None,
                                        op0=ALU.mult)
            B_b = sml.tile([1, GC], BF, tag="B_b", name="B_" + sq_tag)
            nc.vector.tensor_tensor(B_b[:, :], s1b[:, :], rc[:, :], ALU.mult)
            return A_b, B_b, s1b

        def phase_TAILA(g):
            s = ST[g]
            U_sb = s["U_sb"]
            ctx_b16 = []
            for mt in range(2):
                sl = slice(mt * 128, (mt + 1) * 128)
                ps = ps_tile(f"ps_ctx{mt}")
                for kt in range(2):
                    nc.tensor.matmul(ps[:, :], W["w_o"][kt][:, sl],
                                     U_sb[kt][:, :],
                                     start=(kt == 0), stop=(kt == 1))
                cb = act.tile([128, GC], BF, tag=f"ctxb{mt}", name=f"ctxb{mt}")
                nc.scalar.activation(cb[:, :], ps[:, :], ACTF.Identity,
                                     bias=vbias("bo_p", mt), scale=1.0)
                ctx_b16.append(cb)

            A1, B1, s1_b16 = ln_stats(ctx_b16, "sqc")
            ps_A1 = ps_tile("ps_A1rep")
            nc.tensor.matmul(ps_A1[:, :], ones_1_128[:, :], A1[:, :],
                             start=True, stop=True)
            A1rep = act.tile([128, GC], BF, tag="A1rep", name="A1rep")
            nc.scalar.copy(A1rep[:, :], ps_A1[:, :])

            ctxp_f, ctxp_b16 = [], []
            for mt in range(2):
                sl = slice(mt * 128, (mt + 1) * 128)
                ps = ps_tile(f"ps_cp{mt}")
                for kt in range(2):
                    nc.tensor.matmul(ps[:, :], W["w_ctx"][kt][:, sl],
                                     ctx_b16[kt][:, :],
                                     start=(kt == 0), stop=False)
                nc.tensor.matmul(ps[:, :], W["neg_gw"][:, sl], s1_b16[:, :],
                                 start=False, stop=True)
                tmul = act.tile([128, GC], BF, tag=f"cpm{mt}", name=f"cpm{mt}")
                nc.vector.tensor_tensor(tmul[:, :], A1rep[:, :], ps[:, :],
                                        ALU.mult)
                sgc = act.tile([128, GC], FP32, tag="fscr", bufs=3,
                               name=f"sgc{mt}")
                nc.scalar.activation(sgc[:, :], tmul[:, :], ACTF.Sigmoid,
                                     bias=vbias("bw_ctx2", mt), scale=2.0)
                cpf = act.tile([128, GC], FP32, tag=f"ctxp{mt}",
                               name=f"ctxp{mt}")
                nc.vector.tensor_scalar(cpf[:, :], sgc[:, :], 2.0, -1.0,
                                        op0=ALU.mult, op1=ALU.add)
                ctxp_f.append(cpf)
                cpb = act.tile([128, GC], BF, tag=f"ctxpb{mt}",
                               name=f"ctxpb{mt}")
                nc.vector.tensor_copy(cpb[:, :], cpf[:, :])
                ctxp_b16.append(cpb)
            s.update(ctxp_f=ctxp_f, ctxp_b16=ctxp_b16)

        def phase_TAILB(g):
            s = ST[g]
            gru_f, gru_b16 = s["gru_f"], s["gru_b16"]
            ctxp_f, ctxp_b16 = s["ctxp_f"], s["ctxp_b16"]
            neT = s["neT"]
            rows = slice(g * GC, (g + 1) * GC)
            alpha_f = []
            for mt in range(2):
                sl = slice(mt * 128, (mt + 1) * 128)
                ps = ps_tile(f"ps_al{mt}")
                for kt in range(2):
                    nc.tensor.matmul(ps[:, :], W["w_gate"][kt][:, sl],
                                     gru_b16[kt][:, :],
                                     start=(kt == 0), stop=False)
                for kt in range(2):
                    nc.tensor.matmul(ps[:, :], W["w_gate"][2 + kt][:, sl],
                                     ctxp_b16[kt][:, :],
                                     start=False, stop=(kt == 1))
                al = act.tile([128, GC], FP32, tag=f"alpha{mt}",
                              name=f"alpha{mt}")
                nc.scalar.activation(al[:, :], ps[:, :], ACTF.Sigmoid,
                                     bias=vbias("b_gate", mt), scale=1.0)
                alpha_f.append(al)

            blend_f, blend_b16 = [], []
            for mt in range(2):
                d2 = act.tile([128, GC], FP32, tag="fscr", bufs=3,
                              name=f"d2_{mt}")
                nc.vector.tensor_tensor(d2[:, :], ctxp_f[mt][:, :],
                                        gru_f[mt][:, :], ALU.subtract)
                e2 = act.tile([128, GC], FP32, tag="fscr", bufs=3,
                              name=f"e2_{mt}")
                nc.vector.tensor_tensor(e2[:, :], alpha_f[mt][:, :], d2[:, :],
                                        ALU.mult)
                bl = act.tile([128, GC], FP32, tag=f"blend{mt}",
                              name=f"blend{mt}")
                nc.vector.tensor_tensor(bl[:, :], gru_f[mt][:, :], e2[:, :],
                                        ALU.add)
                blend_f.append(bl)
                bb = act.tile([128, GC], BF, tag=f"blendb{mt}",
                              name=f"blendb{mt}")
                nc.vector.tensor_copy(bb[:, :], bl[:, :])
                blend_b16.append(bb)

            A2, B2, _s1b2 = ln_stats(blend_b16, "sqb")
            hcT = []
            for mt in range(2):
                sl = slice(mt * 128, (mt + 1) * 128)
                ps_Ag = ps_tile(f"ps_Ag{mt}")
                nc.tensor.matmul(ps_Ag[:, :], W["g_out_row"][:, sl], A2[:, :],
                                 start=True, stop=True)
                ps_Bg = ps_tile(f"ps_Bg{mt}")
                nc.tensor.matmul(ps_Bg[:, :], W["g_out_row"][:, sl], B2[:, :],
                                 start=True, stop=False)
                nc.tensor.matmul(ps_Bg[:, :], W["neg_beta_out_row"][:, sl],
                                 ones_row[:, :], start=False, stop=True)
                t3 = act.tile([128, GC], FP32, tag="fscr", bufs=3,
                              name=f"t3_{mt}")
                nc.vector.tensor_tensor(t3[:, :], blend_f[mt][:, :],
                                        ps_Ag[:, :], ALU.mult)
                hct = act.tile([128, GC], FP32, tag=f"hcT{mt}", name=f"hcT{mt}")
                nc.vector.tensor_tensor(hct[:, :], t3[:, :], ps_Bg[:, :],
                                        ALU.subtract)
                hcT.append(hct)

            hc_nat = sml.tile([128, NR4 * D], FP32, tag="hcnat", bufs=1,
                              name="hc_nat")
            ne_nat = sml.tile([128, NR4 * D], FP32, tag="nenat", bufs=1,
                              name="ne_nat")
            for r4 in range(NR4):
                ps = ps_tile(f"ps_otr{r4}", [128, D])
                for mt in range(2):
                    nc.tensor.transpose(ps[:, mt * 128:(mt + 1) * 128],
                                        hcT[mt][:, r4 * 128:(r4 + 1) * 128],
                                        ident)
                nc.vector.tensor_copy(hc_nat[:, r4 * D:(r4 + 1) * D],
                                      ps[:, :])
                ps2 = ps_tile(f"ps_otr2{r4}", [128, D])
                for mt in range(2):
                    nc.tensor.transpose(ps2[:, mt * 128:(mt + 1) * 128],
                                        neT[mt][:, r4 * 128:(r4 + 1) * 128],
                                        ident)
                nc.vector.tensor_copy(ne_nat[:, r4 * D:(r4 + 1) * D],
                                      ps2[:, :])
            nc.scalar.dma_start(
                hc_d[rows, :].rearrange("(a p) c -> p a c", p=128),
                hc_nat[:, :])
            nc.scalar.dma_start(
                nm_d[rows, (M - 1) * D:M * D].rearrange("(a p) c -> p a c",
                                                        p=128),
                ne_nat[:, :])

        # interleaved emission: group 1's PE-heavy phases fill group 0's
        # DVE/ACT-heavy tail gaps (and vice versa)
        preload(0)
        nc.sync.dma_start(kb_sb[:, :], kb_d[:, :])
        preload(1)

        # PE warm-up: keep the tensor engine busy (HAM at 8/8) while the
        # initial DMAs stream in; depends only on kf (first, small load).
        ps_w = ps_tile("ps_warm")
        for i in range(50):
            nc.tensor.transpose(ps_w[:, 0:128], ident, ident)
        warm_sink = sml.tile([1, 4], FP32, tag="wsink", name="warm_sink")
        nc.vector.tensor_copy(warm_sink[:, :], ps_w[0:1, 0:4])

        phase_T(0)
        phase_GRU(0)
        phase_L1(0)
        phase_SM(0)
        phase_L2(0)
        phase_T(1)
        phase_TAILA(0)
        phase_TAILB(0)
        phase_GRU(1)
        phase_L1(1)
        phase_SM(1)
        phase_L2(1)
        phase_TAILA(1)
        phase_TAILB(1)

    nc.compile()
    return nc


def _get_kernel():
    if "nc" not in _BUILD_CACHE:
        _BUILD_CACHE["nc"] = build_kernel()
    return _BUILD_CACHE["nc"]


def make_in_maps(inputs):
    w = _prep_weights(inputs)
    x = np.ascontiguousarray(np.asarray(inputs["inputs"], np.float32))
    h = np.ascontiguousarray(np.asarray(inputs["h_prev"], np.float32))
    mem = np.ascontiguousarray(np.asarray(inputs["memory_flat"], np.float32))
    in_maps = []
    for c in range(N_CORES):
        s = slice(c * BS, (c + 1) * BS)
        in_maps.append({"x": x[s], "h": h[s], "mem": mem[s],
                        "kb": w["kb"], "kf": w["kf"]})
    return in_maps


def kernel(**inputs):
    nc = _get_kernel()
    in_maps = make_in_maps(inputs)
    res = run_bass_kernel_spmd(nc, in_maps, core_ids=list(range(N_CORES)))
    h_corr = np.concatenate([res.results[c]["hcorr"] for c in range(N_CORES)],
                            axis=0)
    new_mem = np.concatenate([res.results[c]["newmem"] for c in range(N_CORES)],
                             axis=0)
    return h_corr, new_mem
